# revision 39
# baseline (speedup 1.0000x reference)
"""GCN encoder Bass kernel for 8 TRN2 NeuronCores.

Strategy: nodes are degree-sorted/snake-sharded across the 8 cores (6250 real
+ 22 pad slots each). Each layer: PE transforms the local shard (stationary =
feature-major input tile, moving = weight), ACT scales by dinv + casts to bf16
node-major, remote_dma_broadcast allgathers all shards into every core's SBUF
token buffer, SWDGE dma_gather (two int16 base-offset views) pulls per-edge
source rows feature-major, DVE grouped-reduces them into the aggregation
buffer, then per-chunk dinv-scale + global BN stats (bn_stats/bn_aggr + tiny
stats broadcast) + fused relu-affine on ACT.

Gather pipeline: descriptor generation is the serial bottleneck of the naive
scheme (~6-8ns/column on one Q7 pair), so gather chunks are prepared with
prepare_only=True spread over all 4 SWDGE queues — queue q runs on Q7 core
pair {2q, 2q+1}, giving 4-way parallel desc-gen — while the DRAINS stay
strictly serialized through a chained trigger_dma (concurrent transpose-
gather drains interleave at SDMA packet granularity and corrupt the xbar
stream; single_packet=True hangs the device). Prep lookahead is 8 chunks
(ring holds 3 x 324 descs per queue) to hide the all-8-core instruction
completion latency. Gather idx streams are ordered port-aware in preprocess:
desc k of a piece drains on SDMA engine k%16 and contends for the SBUF read
port of its source partition, so within-slot source order (sum-invariant) is
chosen to spread ports, and pad indices point at free-port partitions of the
zero ranks (~11% drain speedup, HW-measured).
"""
import numpy as np
import ml_dtypes
from contextlib import ExitStack

import concourse.bass as bass
import concourse.bacc as bacc
import concourse.mybir as mybir

N, E, FIN, H, OUT = 50000, 800000, 128, 128, 64
NCORES = 8
SHARD = 6272
REAL = 6250
NT = SHARD // 128          # 49
NSLOT = NCORES * SHARD     # 50176
YN_RANKS = 394             # rank 0 zeros(A) | 392 data | rank 393 zeros(B)
YN_ELEMS = YN_RANKS * 128  # 50432 bf16 per partition
A_BASE = 128
B_SHIFT = 17536
A_MAX_V = 32639
B_MIN_V = 17536
ZB_BASE = 32640
B_VIEW_RANK = 138
GCAP_COLS = 5120
NGBUF = 4                  # G buffers (chunk c drains into G[c % NGBUF])
NQUEUE = 4                 # SWDGE queues == Q7 core pairs generating descs
NQ = 1                     # pieces per chunk = just the A/B view split
TGROUP = 1                 # chunks per trigger group (one ring FIFO burst)
BN_EPS = 1e-5
bf16 = ml_dtypes.bfloat16
f32 = mybir.dt.float32
bfl = mybir.dt.bfloat16
AF = mybir.ActivationFunctionType
AL = mybir.AluOpType


def preprocess(edge_index):
    src = edge_index[0].astype(np.int64)
    dst = edge_index[1].astype(np.int64)
    deg_in = np.bincount(dst, minlength=N)
    deg = (deg_in + 1).astype(np.float64)
    dinv = (1.0 / np.sqrt(deg)).astype(np.float32)

    src_all = np.concatenate([src, np.arange(N)])
    dst_all = np.concatenate([dst, np.arange(N)])
    tot = deg_in + 1

    def assign(order):
        rank = np.arange(N)
        rnd = rank // NCORES
        pos = rank % NCORES
        core_of_rank = np.where(rnd % 2 == 0, pos, NCORES - 1 - pos)
        slot_global = np.empty(N, np.int64)
        node_of_slot = np.full(NSLOT, -1, np.int64)
        for c in range(NCORES):
            nodes_c = order[core_of_rank == c]
            slot_global[nodes_c] = c * SHARD + np.arange(len(nodes_c))
            node_of_slot[c * SHARD + np.arange(len(nodes_c))] = nodes_c
        return slot_global, node_of_slot

    def classify(slot_global):
        sslot = slot_global[src_all]
        na = np.zeros(N, np.int64)
        nb = np.zeros(N, np.int64)
        nm = np.zeros(N, np.int64)
        isa = sslot < B_MIN_V
        isb = sslot > A_MAX_V
        ism = ~isa & ~isb
        np.add.at(na, dst_all[isa], 1)
        np.add.at(nb, dst_all[isb], 1)
        np.add.at(nm, dst_all[ism], 1)
        return na, nb, nm

    order0 = np.argsort(-tot, kind="stable")
    rank = np.arange(N)
    rnd = rank // NCORES
    pos = rank % NCORES
    core_of_rank = np.where(rnd % 2 == 0, pos, NCORES - 1 - pos)
    slot_global, node_of_slot = assign(order0)
    for _ in range(2):
        na, nb, nm = classify(slot_global)
        sg2 = np.empty(N, np.int64)
        ns2 = np.full(NSLOT, -1, np.int64)
        for c in range(NCORES):
            nodes_c = order0[core_of_rank == c]
            k = np.lexsort((-(na[nodes_c] - nb[nodes_c]), -(tot[nodes_c] // 3)))
            nodes_c = nodes_c[k]
            sg2[nodes_c] = c * SHARD + np.arange(len(nodes_c))
            ns2[c * SHARD + np.arange(len(nodes_c))] = nodes_c
        slot_global, node_of_slot = sg2, ns2

    sslot = slot_global[src_all]
    dslot = slot_global[dst_all]
    order_e = np.argsort(dslot, kind="stable")
    sslot_s = sslot[order_e]
    counts = np.bincount(dslot[order_e], minlength=NSLOT)
    starts = np.concatenate([[0], np.cumsum(counts)])

    SA = np.zeros(NT, np.int64)
    SB = np.zeros(NT, np.int64)
    a_lists = [None] * NSLOT
    b_lists = [None] * NSLOT
    for t in range(NT):
        info = []
        for c in range(NCORES):
            for p in range(128):
                s = c * SHARD + t * 128 + p
                nb_ = sslot_s[starts[s]:starts[s + 1]]
                a = nb_[nb_ < B_MIN_V]
                b = nb_[nb_ > A_MAX_V]
                f = nb_[(nb_ >= B_MIN_V) & (nb_ <= A_MAX_V)]
                info.append((s, a, b, f))
        amax = max(len(a) for _, a, _, _ in info)
        afmax = max(len(a) + len(f) for _, a, _, f in info)
        best = None
        for sa_c in range(amax, afmax + 1):
            sb_need = max(len(b) + max(0, len(a) + len(f) - sa_c)
                          for _, a, b, f in info)
            if best is None or sa_c + sb_need < best[0] + best[1]:
                best = (sa_c, sb_need, sa_c)
        sa_e, sb_e, sa_c = best
        SA[t], SB[t] = max(sa_e, 2), max(sb_e, 2)
        for s, a, b, f in info:
            take = min(max(0, sa_c - len(a)), len(f))
            a_lists[s] = np.concatenate([a, f[:take]])
            b_lists[s] = np.concatenate([b, f[take:]])

    chunks = []
    t0 = 0
    cols = 0
    for t in range(NT):
        tc = 128 * (SA[t] + SB[t])
        if t > t0 and cols + tc > GCAP_COLS:
            chunks.append((t0, t))
            t0, cols = t, 0
        cols += tc
    chunks.append((t0, NT))
    gslot_cols = max(sum(128 * (SA[t] + SB[t]) for t in range(a, b))
                     for a, b in chunks)

    # Build per-core idx streams with SBUF-read-port-aware ordering: desc k of
    # a piece drains on SDMA engine k%16, and concurrently-draining descs
    # contend for the port of their source partition (port(p) =
    # ((p%32)//4)*2 + p//64). Within a slot's group any order sums the same,
    # and pad indices may point at ANY partition of the zero ranks — so
    # greedily pick, per position, a source whose port is least-represented in
    # the trailing 15 positions, and spend pads on empty ports. ~10% drain
    # gain (HW-measured on a microbench: 161 -> 180 GB/s).
    PORT = np.array([((p % 32) // 4) * 2 + p // 64 for p in range(128)],
                    np.int64)
    PAD_PART = np.array([(q // 2) * 4 + (q % 2) * 64 for q in range(16)],
                        np.int64)
    total_len = int(sum(128 * (SA[t] + SB[t]) for t in range(NT)))
    idx_streams = []
    for c in range(NCORES):
        stream = np.empty(total_len, np.int64)
        pos = 0
        cnt = [0] * 16
        ring = [-1] * 15
        rpos = 0
        for (ta, tb) in chunks:
            for view in ("A", "B"):
                for t in range(ta, tb):
                    S = int(SA[t] if view == "A" else SB[t])
                    for p in range(128):
                        s = c * SHARD + t * 128 + p
                        if view == "A":
                            srcs = a_lists[s] + A_BASE
                            pad_base = 0
                        else:
                            srcs = b_lists[s] - B_SHIFT
                            pad_base = ZB_BASE
                        k = len(srcs)
                        npad = S - k
                        ports = PORT[srcs % 128].tolist()
                        srcl = srcs.tolist()
                        used = [False] * k
                        for _ in range(S):
                            best = -1
                            bc = 1 << 30
                            for i in range(k):
                                if used[i]:
                                    continue
                                ci = cnt[ports[i]]
                                if ci < bc:
                                    bc, best = ci, i
                                    if ci == 0:
                                        break
                            if npad > 0 and (best < 0 or bc > 0):
                                q = cnt.index(min(cnt))
                                idxv = pad_base + int(PAD_PART[q])
                                pt = q
                                npad -= 1
                            else:
                                used[best] = True
                                idxv = srcl[best]
                                pt = ports[best]
                            stream[pos] = idxv
                            old = ring[rpos]
                            if old >= 0:
                                cnt[old] -= 1
                            ring[rpos] = pt
                            cnt[pt] += 1
                            rpos = (rpos + 1) % 15
                            pos += 1
        assert pos == total_len
        assert stream.min() >= 0 and stream.max() <= 32767
        idx_streams.append(stream.astype(np.int16))

    total_cols = len(idx_streams[0])
    idx_dram = np.zeros((NCORES, 128, total_cols // 16), np.int16)
    for c in range(NCORES):
        w = idx_streams[c].reshape(-1, 16).T
        for g in range(8):
            idx_dram[c, 16 * g:16 * (g + 1), :] = w

    dinv_slot = np.zeros(NSLOT, np.float32)
    m = node_of_slot >= 0
    dinv_slot[m] = dinv[node_of_slot[m]]

    return dict(dinv_slot=dinv_slot, node_of_slot=node_of_slot,
                SA=SA, SB=SB, chunks=chunks, gslot_cols=gslot_cols,
                idx_dram=idx_dram, total_cols=total_cols)


class Sem:
    """semaphore + python-side cumulative counter"""
    def __init__(self, nc, name):
        self.h = nc.alloc_semaphore(name)
        self.n = 0

    def inc(self, inst, k):
        inst.then_inc(self.h, k)
        self.n += k
        return self.n


def build_program(pp, layers=4, do_bcast=True, do_gather=True, do_stats=True, debug_dump=False):
    SA, SB, chunks = pp["SA"], pp["SB"], pp["chunks"]
    gslot_cols = pp["gslot_cols"]
    idx_cols = pp["total_cols"] // 16
    nchunks = len(chunks)
    maxtiles = max(tb - ta for ta, tb in chunks)

    # SWDGE queue q is served by Q7 core pair {2q, 2q+1} (ucode dispatches on
    # cpu_id/2 == queue_num): prepare_only descriptor generation runs on 4
    # core pairs in parallel. Drains stay strictly serialized via a
    # trigger_dma chain (concurrent transpose-gather drains interleave at SDMA
    # packet granularity and corrupt the xbar stream — measured). Chunks are
    # grouped TGROUP-at-a-time on one queue per group: within a group the ring
    # drains FIFO back-to-back with no trigger latency, and the cross-group
    # trigger latency hides behind the tail chunk's drain. Per-queue rings
    # complete FIFO, so drain accounting is per-queue (gd4[q]).
    groups = [(s, min(s + TGROUP, nchunks))
              for s in range(0, nchunks, TGROUP)]
    ngroups = len(groups)
    # greedy load balance over queues (chunk sizes vary ~1.5x): assign each
    # chunk to the least-loaded core pair so no pair's desc-gen falls behind
    # the serialized drain chain. Cap any queue at 3 of the 4-chunk pending
    # window (ring holds 3 prepped chunks max: 3*324 <= 1023 descs).
    chunk_cols = [int(sum(128 * (SA[t] + SB[t]) for t in range(a, b)))
                  for a, b in chunks]
    # Chunks are assigned in RUNS of 3 to one queue: within a run the ring
    # drains FIFO back-to-back (per-engine descriptor order is contiguous per
    # gather, so no xbar interleave — same condition the single-queue baseline
    # relied on), and the cross-run drain-complete wait is paid once per 3
    # chunks. Runs cycle 1,2,3,0,... so each layer's first ~9 chunks avoid
    # queue 0, letting their preps be emitted before the layer boundary
    # without entering the broadcast's queue-0 pending FIFO. Ring capacity:
    # one run = 3*324 descs <= 1023.
    LOOKAHEAD = 2 * NQUEUE
    RUN = 3
    qof_grp = []
    for l in range(layers):
        for g in range(ngroups):
            qof_grp.append((1 + g // RUN) % NQUEUE)
    qof_g = []
    for l in range(layers):
        for gi, (s, e) in enumerate(groups):
            qof_g.extend([qof_grp[l * ngroups + gi]] * (e - s))

    # num_swdge_queues=4 shrinks the per-queue descriptor ring to 1/4 size,
    # which forces the gather desc-gen to flush incremental doorbells —
    # overlapping generation with DMA drain. All traffic stays on queue 0;
    # actually SPREADING instructions across queues corrupts data (measured).
    nc = bacc.Bacc("TRN2", target_bir_lowering=False, debug=False,
                   num_devices=NCORES, num_swdge_queues=4)

    # DRAM I/O
    xT_d = nc.dram_tensor("xT", [128, SHARD], f32, kind="ExternalInput")
    idx_d = nc.dram_tensor("idx", [128, idx_cols], mybir.dt.int16,
                           kind="ExternalInput")
    drep_d = nc.dram_tensor("drep", [128, SHARD], bfl, kind="ExternalInput")
    dnode_d = nc.dram_tensor("dnode", [128, NT], f32, kind="ExternalInput")
    wall_d = nc.dram_tensor("wall", [128, 512], f32, kind="ExternalInput")
    gb_d = nc.dram_tensor("gb", [128, 8], f32, kind="ExternalInput")
    out_d = nc.dram_tensor("out", [64, SHARD], f32, kind="ExternalOutput")
    if debug_dump:
        dbg_stage = nc.dram_tensor("dbg_stage", [128, SHARD], bfl,
                                   kind="ExternalOutput")
        dbg_yn = nc.dram_tensor("dbg_yn", [128, YN_ELEMS], bfl,
                                kind="ExternalOutput")
        dbg_g = nc.dram_tensor("dbg_g", [128, pp["gslot_cols"]], bfl,
                               kind="ExternalOutput")

    ctx = ExitStack()
    # SBUF
    yn = ctx.enter_context(nc.sbuf_tensor([128, YN_ELEMS], bfl))
    idx_sb = ctx.enter_context(nc.sbuf_tensor([128, idx_cols], mybir.dt.int16))
    G = [ctx.enter_context(nc.sbuf_tensor(f"G{i}", [128, gslot_cols], bfl))
         for i in range(NGBUF)]
    acc = ctx.enter_context(nc.sbuf_tensor([128, SHARD], f32))
    drep = ctx.enter_context(nc.sbuf_tensor([128, SHARD], bfl))
    stage = ctx.enter_context(nc.sbuf_tensor([128, SHARD], bfl))

    wsb = ctx.enter_context(nc.sbuf_tensor([128, 512], f32))
    dnode = ctx.enter_context(nc.sbuf_tensor([128, NT], f32))
    gbv = ctx.enter_context(nc.sbuf_tensor([128, 8], f32))
    accA = ctx.enter_context(nc.sbuf_tensor([128, 128], f32))
    accB = ctx.enter_context(nc.sbuf_tensor([128, 128], f32))
    stats6 = ctx.enter_context(nc.sbuf_tensor([128, 6 * max(28, nchunks)], f32))
    mv = ctx.enter_context(nc.sbuf_tensor([128, 8], f32))
    xch_s = ctx.enter_context(nc.sbuf_tensor([128, 2], f32))
    xch_r = ctx.enter_context(nc.sbuf_tensor([128, 16], f32))
    kvec = ctx.enter_context(nc.sbuf_tensor([128, 1], f32))
    bvec = ctx.enter_context(nc.sbuf_tensor([128, 1], f32))
    t0v = ctx.enter_context(nc.sbuf_tensor([128, 1], f32))
    t1v = ctx.enter_context(nc.sbuf_tensor([128, 1], f32))
    t2v = ctx.enter_context(nc.sbuf_tensor([128, 1], f32))
    t2av = ctx.enter_context(nc.sbuf_tensor([128, 1], f32))
    s2v = ctx.enter_context(nc.sbuf_tensor([128, 2], f32))
    # one full 2KB PSUM bank per tile: concurrent PE-write + ACT-read in the
    # same bank is a hardware fault, so never co-locate two tiles in a bank.
    ps_full = [ctx.enter_context(nc.psum_tensor(f"ps{i}", [128, 512], f32))
               for i in range(4)]
    ps = [p[:, 0:128] for p in ps_full]
    ps_dummy = ctx.enter_context(nc.psum_tensor("psd", [128, 512], f32))

    # semaphores
    ld = Sem(nc, "ld"); xs = Sem(nc, "xs"); mm = Sem(nc, "mm")
    ynS = Sem(nc, "ynS"); bn = Sem(nc, "bn")
    gd4 = [Sem(nc, f"gd{q}") for q in range(NQUEUE)]
    pq4 = [Sem(nc, f"pq{q}") for q in range(NQUEUE)]
    gq = Sem(nc, "gq"); rs = Sem(nc, "rs"); ls = Sem(nc, "ls")
    dn = Sem(nc, "dn"); dl = Sem(nc, "dl"); psm = Sem(nc, "psm")
    srs = Sem(nc, "srs"); sls = Sem(nc, "sls"); sqr = Sem(nc, "sqr")
    kb = Sem(nc, "kb"); st = Sem(nc, "st"); sq = Sem(nc, "sq")
    od = Sem(nc, "od"); fv = Sem(nc, "fv"); fa = Sem(nc, "fa")

    # sub-broadcast split points (tiles): each fires as soon as its ACT
    # copies land, overlapping the matmul/ACT pipeline.
    bsplits = [16, 32, NT]

    # per-chunk A/B column counts and idx column offsets
    chunk_meta = []
    icol = 0
    for (ta, tb) in chunks:
        colsA = int(sum(128 * SA[t] for t in range(ta, tb)))
        colsB = int(sum(128 * SB[t] for t in range(ta, tb)))
        chunk_meta.append((ta, tb, colsA, colsB, icol, icol + colsA // 16))
        icol += (colsA + colsB) // 16
    assert icol == idx_cols

    # split each chunk's A+B gather into ~equal column spans: each dma_gather
    # auto-fires its descriptors at generation end, so smaller pieces pipeline
    # desc-gen with DMA drain on its SWDGE ring.
    chunk_pieces = []          # per chunk: list of (p0, p1, isA)
    for (ta, tb, colsA, colsB, ic0, icA) in chunk_meta:
        total = colsA + colsB
        marks = sorted({colsA, total} |
                       {min((q * total // NQ) // 128 * 128, total)
                        for q in range(1, NQ)})
        pieces = []
        p0 = 0
        for m in marks:
            if m > p0:
                pieces.append((p0, m, m <= colsA))
                p0 = m
        chunk_pieces.append(pieces)
    # cumulative drain target per global chunk on its queue (per-queue ring:
    # FIFO completion within a queue only)
    gd_cum = []
    qrun = [0] * NQUEUE
    for c in range(layers * nchunks):
        qrun[qof_g[c]] += 16 * len(chunk_pieces[c % nchunks])
        gd_cum.append(qrun[qof_g[c]])

    with nc.Block() as block:

        @block.sync
        def _(sp):
            for d_, s_ in [(idx_sb, idx_d), (drep, drep_d), (dnode, dnode_d),
                           (wsb, wall_d), (gbv, gb_d)]:
                sp.dma_start(d_[:], s_[:]).then_inc(ld.h, 16)
            ld.n = 80
            # layer-0 input loads straight into acc: the aggregation's first
            # write to acc is ordered after every layer-0 transform read.
            sp.dma_start(acc[:], xT_d[:]).then_inc(xs.h, 16)
            xs.n += 16
            if debug_dump:
                sp.wait_ge(kb.h, layers)
                if do_stats:
                    sp.wait_ge(sqr.h, min(layers, 3))
                sp.dma_start(dbg_stage[:], stage[:]).then_inc(od.h, 16)
                od.n += 16
                sp.dma_start(dbg_yn[:], yn[:]).then_inc(od.h, 16)
                od.n += 16
                with nc.allow_non_contiguous_dma(reason="debug dumps"):
                    for j, src_ap in enumerate([xch_r[:], xch_s[:], mv[:],
                                                kvec[:], bvec[:], t0v[:],
                                                t1v[:], s2v[:], stats6[:]]):
                        w = src_ap.shape[1]
                        sp.dma_start(dbg_g.bitcast(f32)[:, 40*j:40*j+w],
                                     src_ap).then_inc(od.h, 16)
                        od.n += 16
            sp.wait_ge(bn.h, layers if (do_stats and layers == 4) else 0)
            if not (do_stats and layers == 4):
                sp.wait_ge(kb.h, layers)
            sp.dma_start(out_d[:], acc[0:64, :]).then_inc(od.h, 16)
            od.n += 16
            sp.wait_ge(od.h, od.n)

        @block.tensor
        def _(te):
            te.wait_ge(ld.h, 80)
            for l in range(layers):
                for t in range(NT):
                    i = l * NT + t
                    if l == 0:
                        if t == 0:
                            te.wait_ge(xs.h, 16)
                        lhsT = acc[:, t * 128:(t + 1) * 128]
                    else:
                        if t == 0:
                            te.wait_ge(bn.h, l)
                        lhsT = acc[:, t * 128:(t + 1) * 128]
                    if i >= 4:
                        te.wait_ge(ynS.h, i - 3)
                    nc.tensor.matmul(
                        ps[i % 4], lhsT,
                        wsb[:, l * 128:(l + 1) * 128],
                        start=True, stop=True,
                    ).then_inc(mm.h, 1)
                    mm.n += 1
                # two per-layer dummy matmuls: the ACT copy of tile i waits
                # mm >= i+2 (PE drain provably complete); the layer's last
                # tiles need successors that don't depend on later layers.
                for _ in range(2):
                    nc.tensor.matmul(
                        ps_dummy[:, 0:128], wsb[:, 0:128], wsb[:, 0:128],
                        start=True, stop=True,
                    ).then_inc(mm.h, 1)
                    mm.n += 1

        @block.scalar
        def _(sc):
            sc.wait_ge(ld.h, 80)
            for l in range(layers):
                for t in range(NT):
                    i = l * NT + t
                    sc.wait_ge(mm.h, l * (NT + 2) + t + 2)
                    if l >= 1 and t == 0:
                        sc.wait_ge(ls.h, 16 * len(bsplits) * l)
                    sc.activation(
                        stage[:, t * 128:(t + 1) * 128], ps[i % 4],
                        AF.Copy, bias=0.0, scale=dnode[:, t:t + 1],
                    ).then_inc(ynS.h, 1)
                    ynS.n += 1
                if not do_stats:
                    continue
                if l < 3:
                    sc.wait_ge(sq.h, l + 1)
                    sc.activation(t1v[:], t0v[:], AF.Sqrt).then_inc(fa.h, 1)
                    fa.n += 1
                    sc.wait_ge(fa.h, fa.n)
                    # readback after fence: t1v committed before sqr fires
                    sc.activation(t2av[:], t1v[:], AF.Copy).then_inc(sqr.h, 1)
                    sqr.n += 1
                    if debug_dump and l == layers - 1:
                        continue
                    sc.wait_ge(kb.h, l + 1)
                    sc.activation(acc[:], acc[:], AF.Relu,
                                  bias=bvec[:], scale=kvec[:],
                                  ).then_inc(bn.h, 1)
                else:
                    sc.wait_ge(kb.h, l + 1)
                    sc.activation(acc[:], acc[:], AF.Identity,
                                  bias=gbv[:, 6:7], scale=1.0,
                                  ).then_inc(bn.h, 1)
                bn.n += 1

        @block.vector
        def _(ve):
            ve.wait_ge(ld.h, 80)
            cidx = 0
            for l in range(layers):
                # Small (4-8B/partition) DVE writes commit lazily: a
                # dependent read in the very next op sees stale data.
                # Fence each small write with a self-semaphore wait.
                def ff(inst):
                    inst.then_inc(fv.h, 1)
                    fv.n += 1
                    ve.wait_ge(fv.h, fv.n)
                # NOTE: no 2-port-perf-mode DVE ops (tensor_tensor with a
                # bf16 operand, bn_stats on f32) may run inside this loop:
                # they grab the shared SBUF port pair that GPSIMD needs to
                # write SWDGE gather descriptors, starving the gather DMA.
                # fp32 tensor_reduce / fp32+fp32 tensor_tensor are 1x-mode
                # (dedicated ports only) and safe.
                for ci, (ta, tb, colsA, colsB, ic0, icA) in enumerate(chunk_meta):
                    if not do_gather:
                        break
                    ve.wait_ge(gd4[qof_g[cidx]].h, gd_cum[cidx])
                    g = G[cidx % NGBUF]
                    offA = 0
                    offB = int(sum(128 * SA[t] for t in range(ta, tb)))
                    for t in range(ta, tb):
                        wA = 128 * int(SA[t])
                        wB = 128 * int(SB[t])
                        ve.tensor_reduce(
                            out=accA[:],
                            in_=g[:, offA:offA + wA].rearrange(
                                "p (n s) -> p n s", n=128),
                            axis=mybir.AxisListType.X, op=AL.add)
                        ve.tensor_reduce(
                            out=accB[:],
                            in_=g[:, offB:offB + wB].rearrange(
                                "p (n s) -> p n s", n=128),
                            axis=mybir.AxisListType.X, op=AL.add)
                        offA += wA
                        offB += wB
                        tt = ve.tensor_tensor(
                            out=acc[:, t * 128:(t + 1) * 128],
                            in0=accA[:],
                            in1=accB[:], op=AL.add)
                    tt.then_inc(gq.h, 1)
                    gq.n += 1
                    # per-chunk dinv_dst scale + BN stats window: pulls this
                    # work off the serial end-of-layer tail, overlapping the
                    # remaining chunks' drains.
                    c0, c1 = ta * 128, tb * 128
                    last_dmul = ve.tensor_tensor(
                        out=acc[:, c0:c1], in0=acc[:, c0:c1],
                        in1=drep[:, c0:c1], op=AL.mult)
                    if do_stats and l < 3:
                        ins_ = ve.bn_stats(stats6[:, ci * 6:(ci + 1) * 6],
                                           acc[:, c0:min(c1, REAL)])
                    cidx += 1
                if do_stats and l < 3:
                    ff(ins_)
                    ff(ve.bn_aggr(mv[:, 0:2], stats6[:, 0:6 * nchunks]))
                    # xch_s = [mean, mean^2 + var] = [Ex, Ex2]
                    if l > 0:
                        ve.wait_ge(sls.h, 16 * l)  # prev stats send done
                    ve.tensor_copy(xch_s[:, 0:1], mv[:, 0:1])
                    ff(ve.tensor_tensor(out=t2v[:], in0=mv[:, 0:1],
                                        in1=mv[:, 0:1], op=AL.mult))
                    ff(ve.tensor_tensor(out=xch_s[:, 1:2], in0=mv[:, 1:2],
                                        in1=t2v[:], op=AL.add))
                    # readback signals xch_s committed
                    ve.tensor_copy(t2v[:], xch_s[:, 0:1]).then_inc(st.h, 1)
                    st.n += 1
                    ve.wait_ge(srs.h, 16 * (l + 1))
                    # global stats: average 8 partials
                    ff(ve.tensor_reduce(
                        out=s2v[:],
                        in_=xch_r[:].rearrange("p (c k) -> p k c", c=8),
                        axis=mybir.AxisListType.X, op=AL.add))
                    ff(ve.tensor_scalar(out=s2v[:], in0=s2v[:],
                                        scalar1=0.125, scalar2=None,
                                        op0=AL.mult))
                    # var = Ex2m - gmean^2 + eps ; t0 = 1/var
                    ff(ve.tensor_tensor(out=t2v[:], in0=s2v[:, 0:1],
                                        in1=s2v[:, 0:1], op=AL.mult))
                    ff(ve.tensor_tensor(out=t0v[:], in0=s2v[:, 1:2],
                                        in1=t2v[:], op=AL.subtract))
                    ff(ve.tensor_scalar(out=t0v[:], in0=t0v[:],
                                        scalar1=BN_EPS, scalar2=None,
                                        op0=AL.add))
                    ff(ve.reciprocal(t0v[:], t0v[:]))
                    ve.tensor_copy(t2v[:], t0v[:]).then_inc(sq.h, 1)
                    sq.n += 1
                    # ACT computes t1 = sqrt(t0) = rstd
                    ve.wait_ge(sqr.h, l + 1)
                    ff(ve.tensor_tensor(out=kvec[:],
                                        in0=gbv[:, 2 * l:2 * l + 1],
                                        in1=t1v[:], op=AL.mult))
                    ff(ve.tensor_tensor(out=t2v[:], in0=s2v[:, 0:1],
                                        in1=kvec[:], op=AL.mult))
                    ff(ve.tensor_tensor(out=bvec[:],
                                        in0=gbv[:, 2 * l + 1:2 * l + 2],
                                        in1=t2v[:], op=AL.subtract))
                    ve.tensor_copy(t2v[:], bvec[:]).then_inc(kb.h, 1)
                else:
                    last_dmul.then_inc(kb.h, 1)
                kb.n += 1

        @block.gpsimd
        def _(gp):
            gp.wait_ge(ld.h, 80)
            gp.memset(yn[:, 0:128], 0)
            gp.memset(yn[:, B_VIEW_RANK * 128 + 32768 - 128:
                          B_VIEW_RANK * 128 + 32768], 0)
            nch = nchunks
            prep_tgt = {}
            npieces = {}

            def emit_prep(li, ci):
                cdx = li * nch + ci
                q = qof_g[cdx]
                g = G[cdx % NGBUF]
                (ta, tb, colsA, colsB, ic0, icA) = chunk_meta[ci]
                cnt = 0
                for (p0, p1, isA) in chunk_pieces[ci]:
                    if isA:
                        in_ap = yn[:, 0:32768]
                        i0 = ic0 + p0 // 16
                    else:
                        in_ap = yn[:, B_VIEW_RANK * 128:
                                   B_VIEW_RANK * 128 + 32768]
                        i0 = icA + (p0 - colsA) // 16
                    w = p1 - p0
                    gp.dma_gather(
                        out_ap=g[:, p0:p1].rearrange(
                            "p (o n) -> p o n", o=1),
                        in_ap=in_ap,
                        idxs_ap=idx_sb[:, i0:i0 + w // 16],
                        num_idxs=w, num_idxs_reg=w,
                        elem_size=128, transpose=True,
                        sbuf_tokens_per_rank=128,
                        sbuf_free_dim_per_rank=256,
                        single_packet=False,
                        queue_num=q,
                        prepare_only=True, sem=gd4[q].h,
                    ).then_inc(pq4[q].h, 1)
                    pq4[q].n += 1
                    gd4[q].n += 16
                    cnt += 1
                prep_tgt[cdx] = pq4[q].n
                npieces[cdx] = cnt

            def emit_trigger(li, ci):
                cdx = li * nch + ci
                q = qof_g[cdx]
                gp.wait_ge(pq4[q].h, prep_tgt[cdx])
                if cdx >= NGBUF:
                    gp.wait_ge(gq.h, cdx - (NGBUF - 1))
                # same-queue predecessor drains FIFO in the same ring —
                # back-to-back, no completion wait needed; only cross-queue
                # transitions must serialize on the drain semaphore.
                if cdx >= 1 and qof_g[cdx - 1] != q:
                    gp.wait_ge(gd4[qof_g[cdx - 1]].h, gd_cum[cdx - 1])
                gp.trigger_dma(count=npieces[cdx], queue_num=q)

            for l in range(layers):
                if l > 0:
                    gp.wait_ge(dn.h, 16 * l)
                prev = 0
                for bs in bsplits:
                    gp.wait_ge(ynS.h, NT * l + bs)
                    ynoff = gp.partition_id() * SHARD + (128 + prev * 128)
                    gp.remote_dma_broadcast(
                        out_ap=yn[:, bass.ds(ynoff, (bs - prev) * 128)],
                        in_ap=stage[:, prev * 128:bs * 128],
                        remote_sem=rs.h, local_sem=ls.h,
                        rdests=[(0, k) for k in range(NCORES)],
                    ).then_inc(psm.h, 1)
                    psm.n += 1
                    gp.wait_ge(psm.h, psm.n)
                    gp.trigger_dma(count=1)
                    prev = bs
                if do_gather:
                    # layer 0's first preps emit here; later layers' were
                    # already emitted at the end of the previous layer so
                    # their desc-gen hides under the layer boundary.
                    if l == 0:
                        for ci in range(min(LOOKAHEAD, nch)):
                            emit_prep(0, ci)
                    gp.wait_ge(rs.h, 16 * len(bsplits) * (l + 1))
                    for ci in range(nch):
                        emit_trigger(l, ci)
                        if ci + LOOKAHEAD < nch:
                            emit_prep(l, ci + LOOKAHEAD)
                    # this layer's drain totals, captured BEFORE the next
                    # layer's early preps bump the counters (their sems only
                    # fire after the next layer's triggers).
                    gd_tgt = [gd4[q].n for q in range(NQUEUE)]
                    if l + 1 < layers:
                        for ci in range(min(LOOKAHEAD, nch)):
                            emit_prep(l + 1, ci)
                else:
                    gp.wait_ge(rs.h, 16 * len(bsplits) * (l + 1))
                    gd_tgt = [gd4[q].n for q in range(NQUEUE)]
                for q in range(NQUEUE):
                    gp.wait_ge(gd4[q].h, gd_tgt[q])
                gp.remote_sem_update_broadcast(
                    remote_sem=dn.h, local_sem=dl.h,
                    rdests=[(0, k) for k in range(NCORES)],
                ).then_inc(psm.h, 1)
                psm.n += 1
                gp.wait_ge(psm.h, psm.n)
                gp.trigger_dma(count=1)
                if do_stats and l < 3:
                    gp.wait_ge(st.h, l + 1)
                    xoff = gp.partition_id() * 2
                    gp.remote_dma_broadcast(
                        out_ap=xch_r[:, bass.ds(xoff, 2)],
                        in_ap=xch_s[:],
                        remote_sem=srs.h, local_sem=sls.h,
                        rdests=[(0, k) for k in range(NCORES)],
                    ).then_inc(psm.h, 1)
                    psm.n += 1
                    gp.wait_ge(psm.h, psm.n)
                    gp.trigger_dma(count=1)

    nc.compile()
    return nc


def make_core_inputs(pp, x, Ws, gb):
    """per-core in_maps for run_bass_kernel_spmd / run_bass_via_pjrt"""
    nos = pp["node_of_slot"]
    dinv_slot = pp["dinv_slot"]
    wall = np.zeros((128, 512), np.float32)
    wall[:, 0:128] = Ws[0]
    wall[:, 128:256] = Ws[1]
    wall[:, 256:384] = Ws[2]
    wall[:, 384:448] = Ws[3][:, :64] if Ws[3].shape[1] == 64 else Ws[3][:, :]
    in_maps = []
    for c in range(NCORES):
        slots = c * SHARD + np.arange(SHARD)
        nodes = nos[slots]
        msk = nodes >= 0
        xT = np.zeros((128, SHARD), np.float32)
        xT[:, msk] = x[nodes[msk]].T
        drep = np.broadcast_to(
            dinv_slot[slots].astype(bf16), (128, SHARD)).copy()
        dnode = dinv_slot[slots].reshape(NT, 128).T.copy().astype(np.float32)
        in_maps.append(dict(xT=xT, idx=pp["idx_dram"][c].copy(),
                            drep=drep, dnode=dnode, wall=wall.copy(),
                            gb=gb.copy()))
    return in_maps


def make_gb(g1, be1, g2, be2, g3, be3, b4):
    gb = np.zeros((128, 8), np.float32)
    for i, v in enumerate([g1, be1, g2, be2, g3, be3]):
        gb[:, i] = v
    gb[:64, 6] = b4
    return gb


def assemble_output(pp, results):
    nos = pp["node_of_slot"]
    full = np.zeros((N, OUT), np.float32)
    for c in range(NCORES):
        slots = c * SHARD + np.arange(SHARD)
        nodes = nos[slots]
        msk = nodes >= 0
        full[nodes[msk]] = results[c]["out"][:OUT, msk].T
    return full


# ---------------------------------------------------------------------------
# public entry point
# ---------------------------------------------------------------------------
_CACHE = {}


def _get_program(edge_index):
    key = hash(edge_index.tobytes())
    if key not in _CACHE:
        pp = preprocess(edge_index)
        nc = build_program(pp)
        _CACHE[key] = (pp, nc)
    return _CACHE[key]


def kernel(**inputs):
    """Full GCN encoder on 8 TRN2 NeuronCores.

    Takes the full (unsharded) inputs of reference.setup_inputs(), returns
    the full [50000, 64] float32 output.
    """
    from concourse import bass2jax

    inputs = {k: np.asarray(v) for k, v in inputs.items()}
    edge_index = inputs["edge_index"].astype(np.int32)
    pp, nc = _get_program(edge_index)
    Ws = [inputs["W1"], inputs["W2"], inputs["W3"], inputs["W4"]]
    gb = make_gb(inputs["g1"], inputs["be1"], inputs["g2"], inputs["be2"],
                 inputs["g3"], inputs["be3"], inputs["b4"])
    # bias handling: b1..b3 cancel inside batch-norm (per-feature constant
    # shifts drop out of x - mean); b4 is applied on-device via gb col 6.
    in_maps = make_core_inputs(pp, inputs["x"].astype(np.float32), Ws, gb)
    results = bass2jax.run_bass_via_pjrt(nc, in_maps, n_cores=NCORES)
    return assemble_output(pp, results)



# revision 43
# speedup vs baseline: 1.1688x; 1.1688x over previous
"""GCN encoder Bass kernel for 8 TRN2 NeuronCores.

Strategy: nodes are degree-sorted/snake-sharded across the 8 cores (6250 real
+ 22 pad slots each). Each layer: PE transforms the local shard (stationary =
feature-major input tile, moving = weight), ACT scales by dinv + casts to bf16
node-major, remote_dma_broadcast allgathers all shards into every core's SBUF
token buffer, SWDGE dma_gather (two int16 base-offset views) pulls per-edge
source rows feature-major, DVE grouped-reduces them into the aggregation
buffer, then per-chunk dinv-scale + global BN stats (bn_stats/bn_aggr + tiny
stats broadcast) + fused relu-affine on ACT.

Gather pipeline: descriptor generation is the serial bottleneck of the naive
scheme (~6-8ns/column on one Q7 pair), so gather chunks are prepared with
prepare_only=True spread over all 4 SWDGE queues — queue q runs on Q7 core
pair {2q, 2q+1}, giving 4-way parallel desc-gen — while the DRAINS stay
strictly serialized through a chained trigger_dma (concurrent transpose-
gather drains interleave at SDMA packet granularity and corrupt the xbar
stream; single_packet=True hangs the device). Prep lookahead is 8 chunks
(ring holds 3 x 324 descs per queue) to hide the all-8-core instruction
completion latency. Gather idx streams are ordered port-aware in preprocess:
desc k of a piece drains on SDMA engine k%16 and contends for the SBUF read
port of its source partition, so within-slot source order (sum-invariant) is
chosen to spread ports, and pad indices point at free-port partitions of the
zero ranks (~11% drain speedup, HW-measured).
"""
import numpy as np
import ml_dtypes
from contextlib import ExitStack

import concourse.bass as bass
import concourse.bacc as bacc
import concourse.mybir as mybir

N, E, FIN, H, OUT = 50000, 800000, 128, 128, 64
NCORES = 8
SHARD = 6272
REAL = 6250
NT = SHARD // 128          # 49
NSLOT = NCORES * SHARD     # 50176
YN_RANKS = 394             # rank 0 zeros(A) | 392 data | rank 393 zeros(B)
YN_ELEMS = YN_RANKS * 128  # 50432 bf16 per partition
A_BASE = 128
B_SHIFT = 17536
A_MAX_V = 32639
B_MIN_V = 17536
ZB_BASE = 32640
B_VIEW_RANK = 138
GCAP_COLS = 5120
NGBUF = 4                  # G buffers (chunk c drains into G[c % NGBUF])
NQUEUE = 4                 # SWDGE queues == Q7 core pairs generating descs
NQ = 1                     # pieces per chunk = just the A/B view split
TGROUP = 1                 # chunks per trigger group (one ring FIFO burst)
BN_EPS = 1e-5
bf16 = ml_dtypes.bfloat16
f32 = mybir.dt.float32
bfl = mybir.dt.bfloat16
AF = mybir.ActivationFunctionType
AL = mybir.AluOpType


def preprocess(edge_index):
    src = edge_index[0].astype(np.int64)
    dst = edge_index[1].astype(np.int64)
    deg_in = np.bincount(dst, minlength=N)
    deg = (deg_in + 1).astype(np.float64)
    dinv = (1.0 / np.sqrt(deg)).astype(np.float32)

    src_all = np.concatenate([src, np.arange(N)])
    dst_all = np.concatenate([dst, np.arange(N)])
    tot = deg_in + 1

    def assign(order):
        rank = np.arange(N)
        rnd = rank // NCORES
        pos = rank % NCORES
        core_of_rank = np.where(rnd % 2 == 0, pos, NCORES - 1 - pos)
        slot_global = np.empty(N, np.int64)
        node_of_slot = np.full(NSLOT, -1, np.int64)
        for c in range(NCORES):
            nodes_c = order[core_of_rank == c]
            slot_global[nodes_c] = c * SHARD + np.arange(len(nodes_c))
            node_of_slot[c * SHARD + np.arange(len(nodes_c))] = nodes_c
        return slot_global, node_of_slot

    def classify(slot_global):
        sslot = slot_global[src_all]
        na = np.zeros(N, np.int64)
        nb = np.zeros(N, np.int64)
        nm = np.zeros(N, np.int64)
        isa = sslot < B_MIN_V
        isb = sslot > A_MAX_V
        ism = ~isa & ~isb
        np.add.at(na, dst_all[isa], 1)
        np.add.at(nb, dst_all[isb], 1)
        np.add.at(nm, dst_all[ism], 1)
        return na, nb, nm

    order0 = np.argsort(-tot, kind="stable")
    rank = np.arange(N)
    rnd = rank // NCORES
    pos = rank % NCORES
    core_of_rank = np.where(rnd % 2 == 0, pos, NCORES - 1 - pos)
    slot_global, node_of_slot = assign(order0)
    for _ in range(2):
        na, nb, nm = classify(slot_global)
        sg2 = np.empty(N, np.int64)
        ns2 = np.full(NSLOT, -1, np.int64)
        for c in range(NCORES):
            nodes_c = order0[core_of_rank == c]
            k = np.lexsort((-(na[nodes_c] - nb[nodes_c]), -(tot[nodes_c] // 3)))
            nodes_c = nodes_c[k]
            sg2[nodes_c] = c * SHARD + np.arange(len(nodes_c))
            ns2[c * SHARD + np.arange(len(nodes_c))] = nodes_c
        slot_global, node_of_slot = sg2, ns2

    sslot = slot_global[src_all]
    dslot = slot_global[dst_all]
    order_e = np.argsort(dslot, kind="stable")
    sslot_s = sslot[order_e]
    counts = np.bincount(dslot[order_e], minlength=NSLOT)
    starts = np.concatenate([[0], np.cumsum(counts)])

    SA = np.zeros(NT, np.int64)
    SB = np.zeros(NT, np.int64)
    a_lists = [None] * NSLOT
    b_lists = [None] * NSLOT
    for t in range(NT):
        info = []
        for c in range(NCORES):
            for p in range(128):
                s = c * SHARD + t * 128 + p
                nb_ = sslot_s[starts[s]:starts[s + 1]]
                a = nb_[nb_ < B_MIN_V]
                b = nb_[nb_ > A_MAX_V]
                f = nb_[(nb_ >= B_MIN_V) & (nb_ <= A_MAX_V)]
                info.append((s, a, b, f))
        amax = max(len(a) for _, a, _, _ in info)
        afmax = max(len(a) + len(f) for _, a, _, f in info)
        best = None
        for sa_c in range(amax, afmax + 1):
            sb_need = max(len(b) + max(0, len(a) + len(f) - sa_c)
                          for _, a, b, f in info)
            if best is None or sa_c + sb_need < best[0] + best[1]:
                best = (sa_c, sb_need, sa_c)
        sa_e, sb_e, sa_c = best
        SA[t], SB[t] = max(sa_e, 2), max(sb_e, 2)
        for s, a, b, f in info:
            take = min(max(0, sa_c - len(a)), len(f))
            a_lists[s] = np.concatenate([a, f[:take]])
            b_lists[s] = np.concatenate([b, f[take:]])

    chunks = []
    t0 = 0
    cols = 0
    for t in range(NT):
        tc = 128 * (SA[t] + SB[t])
        if t > t0 and cols + tc > GCAP_COLS:
            chunks.append((t0, t))
            t0, cols = t, 0
        cols += tc
    chunks.append((t0, NT))
    gslot_cols = max(sum(128 * (SA[t] + SB[t]) for t in range(a, b))
                     for a, b in chunks)

    # Build per-core idx streams with SBUF-read-port-aware ordering: desc k of
    # a piece drains on SDMA engine k%16, and concurrently-draining descs
    # contend for the port of their source partition (port(p) =
    # ((p%32)//4)*2 + p//64). Within a slot's group any order sums the same,
    # and pad indices may point at ANY partition of the zero ranks — so
    # greedily pick, per position, a source whose port is least-represented in
    # the trailing 15 positions, and spend pads on empty ports. ~10% drain
    # gain (HW-measured on a microbench: 161 -> 180 GB/s).
    PORT = np.array([((p % 32) // 4) * 2 + p // 64 for p in range(128)],
                    np.int64)
    PAD_PART = np.array([(q // 2) * 4 + (q % 2) * 64 for q in range(16)],
                        np.int64)
    total_len = int(sum(128 * (SA[t] + SB[t]) for t in range(NT)))
    idx_streams = []
    for c in range(NCORES):
        stream = np.empty(total_len, np.int64)
        pos = 0
        cnt = [0] * 16
        ring = [-1] * 15
        rpos = 0
        for (ta, tb) in chunks:
            for view in ("A", "B"):
                for t in range(ta, tb):
                    S = int(SA[t] if view == "A" else SB[t])
                    for p in range(128):
                        s = c * SHARD + t * 128 + p
                        if view == "A":
                            srcs = a_lists[s] + A_BASE
                            pad_base = 0
                        else:
                            srcs = b_lists[s] - B_SHIFT
                            pad_base = ZB_BASE
                        k = len(srcs)
                        npad = S - k
                        ports = PORT[srcs % 128].tolist()
                        srcl = srcs.tolist()
                        used = [False] * k
                        for _ in range(S):
                            best = -1
                            bc = 1 << 30
                            for i in range(k):
                                if used[i]:
                                    continue
                                ci = cnt[ports[i]]
                                if ci < bc:
                                    bc, best = ci, i
                                    if ci == 0:
                                        break
                            if npad > 0 and (best < 0 or bc > 0):
                                q = cnt.index(min(cnt))
                                idxv = pad_base + int(PAD_PART[q])
                                pt = q
                                npad -= 1
                            else:
                                used[best] = True
                                idxv = srcl[best]
                                pt = ports[best]
                            stream[pos] = idxv
                            old = ring[rpos]
                            if old >= 0:
                                cnt[old] -= 1
                            ring[rpos] = pt
                            cnt[pt] += 1
                            rpos = (rpos + 1) % 15
                            pos += 1
        assert pos == total_len
        assert stream.min() >= 0 and stream.max() <= 32767
        idx_streams.append(stream.astype(np.int16))

    total_cols = len(idx_streams[0])
    idx_dram = np.zeros((NCORES, 128, total_cols // 16), np.int16)
    for c in range(NCORES):
        w = idx_streams[c].reshape(-1, 16).T
        for g in range(8):
            idx_dram[c, 16 * g:16 * (g + 1), :] = w

    dinv_slot = np.zeros(NSLOT, np.float32)
    m = node_of_slot >= 0
    dinv_slot[m] = dinv[node_of_slot[m]]

    return dict(dinv_slot=dinv_slot, node_of_slot=node_of_slot,
                SA=SA, SB=SB, chunks=chunks, gslot_cols=gslot_cols,
                idx_dram=idx_dram, total_cols=total_cols)


class Sem:
    """semaphore + python-side cumulative counter"""
    def __init__(self, nc, name):
        self.h = nc.alloc_semaphore(name)
        self.n = 0

    def inc(self, inst, k):
        inst.then_inc(self.h, k)
        self.n += k
        return self.n


def build_program(pp, layers=4, do_bcast=True, do_gather=True, do_stats=True, debug_dump=False):
    SA, SB, chunks = pp["SA"], pp["SB"], pp["chunks"]
    gslot_cols = pp["gslot_cols"]
    idx_cols = pp["total_cols"] // 16
    nchunks = len(chunks)
    maxtiles = max(tb - ta for ta, tb in chunks)

    # SWDGE queue q is served by Q7 core pair {2q, 2q+1} (ucode dispatches on
    # cpu_id/2 == queue_num): prepare_only descriptor generation runs on 4
    # core pairs in parallel. Drains stay strictly serialized via a
    # trigger_dma chain (concurrent transpose-gather drains interleave at SDMA
    # packet granularity and corrupt the xbar stream — measured). Chunks are
    # grouped TGROUP-at-a-time on one queue per group: within a group the ring
    # drains FIFO back-to-back with no trigger latency, and the cross-group
    # trigger latency hides behind the tail chunk's drain. Per-queue rings
    # complete FIFO, so drain accounting is per-queue (gd4[q]).
    groups = [(s, min(s + TGROUP, nchunks))
              for s in range(0, nchunks, TGROUP)]
    ngroups = len(groups)
    # greedy load balance over queues (chunk sizes vary ~1.5x): assign each
    # chunk to the least-loaded core pair so no pair's desc-gen falls behind
    # the serialized drain chain. Cap any queue at 3 of the 4-chunk pending
    # window (ring holds 3 prepped chunks max: 3*324 <= 1023 descs).
    chunk_cols = [int(sum(128 * (SA[t] + SB[t]) for t in range(a, b)))
                  for a, b in chunks]
    # prep lookahead is LOOKAHEAD chunks; cap any queue at 3 chunks within
    # the pending window so the ring (1023 descs) never overflows.
    LOOKAHEAD = 2 * NQUEUE
    qof_grp = []
    qload = [0.0] * NQUEUE
    for c in range(layers * ngroups):
        # each layer's first LOOKAHEAD chunks avoid queue 0 so their preps
        # can be emitted at the END of the previous layer (hiding desc-gen
        # under the layer boundary) without entering the broadcast's queue-0
        # pending FIFO.
        allowed = (range(1, NQUEUE) if (c % ngroups) < LOOKAHEAD
                   else range(NQUEUE))
        cand = sorted(allowed, key=lambda q: qload[q])
        q = next(q_ for q_ in cand
                 if qof_grp[-(LOOKAHEAD - 1):].count(q_) < 3)
        qof_grp.append(q)
        qload[q] += chunk_cols[c % nchunks]
    qof_g = []
    for l in range(layers):
        for gi, (s, e) in enumerate(groups):
            qof_g.extend([qof_grp[l * ngroups + gi]] * (e - s))

    # num_swdge_queues=4 shrinks the per-queue descriptor ring to 1/4 size,
    # which forces the gather desc-gen to flush incremental doorbells —
    # overlapping generation with DMA drain. All traffic stays on queue 0;
    # actually SPREADING instructions across queues corrupts data (measured).
    nc = bacc.Bacc("TRN2", target_bir_lowering=False, debug=False,
                   num_devices=NCORES, num_swdge_queues=4)

    # DRAM I/O
    xT_d = nc.dram_tensor("xT", [128, SHARD], f32, kind="ExternalInput")
    idx_d = nc.dram_tensor("idx", [128, idx_cols], mybir.dt.int16,
                           kind="ExternalInput")
    drep_d = nc.dram_tensor("drep", [128, SHARD], bfl, kind="ExternalInput")
    dnode_d = nc.dram_tensor("dnode", [128, NT], f32, kind="ExternalInput")
    wall_d = nc.dram_tensor("wall", [128, 512], f32, kind="ExternalInput")
    gb_d = nc.dram_tensor("gb", [128, 8], f32, kind="ExternalInput")
    out_d = nc.dram_tensor("out", [64, SHARD], f32, kind="ExternalOutput")
    if debug_dump:
        dbg_stage = nc.dram_tensor("dbg_stage", [128, SHARD], bfl,
                                   kind="ExternalOutput")
        dbg_yn = nc.dram_tensor("dbg_yn", [128, YN_ELEMS], bfl,
                                kind="ExternalOutput")
        dbg_g = nc.dram_tensor("dbg_g", [128, pp["gslot_cols"]], bfl,
                               kind="ExternalOutput")

    ctx = ExitStack()
    # SBUF
    yn = ctx.enter_context(nc.sbuf_tensor([128, YN_ELEMS], bfl))
    idx_sb = ctx.enter_context(nc.sbuf_tensor([128, idx_cols], mybir.dt.int16))
    G = [ctx.enter_context(nc.sbuf_tensor(f"G{i}", [128, gslot_cols], bfl))
         for i in range(NGBUF)]
    acc = ctx.enter_context(nc.sbuf_tensor([128, SHARD], f32))
    drep = ctx.enter_context(nc.sbuf_tensor([128, SHARD], bfl))
    stage = ctx.enter_context(nc.sbuf_tensor([128, SHARD], bfl))

    wsb = ctx.enter_context(nc.sbuf_tensor([128, 512], f32))
    dnode = ctx.enter_context(nc.sbuf_tensor([128, NT], f32))
    gbv = ctx.enter_context(nc.sbuf_tensor([128, 8], f32))
    accA = ctx.enter_context(nc.sbuf_tensor([128, 128], f32))
    accB = ctx.enter_context(nc.sbuf_tensor([128, 128], f32))
    stats6 = ctx.enter_context(nc.sbuf_tensor([128, 6 * max(28, nchunks)], f32))
    mv = ctx.enter_context(nc.sbuf_tensor([128, 8], f32))
    xch_s = ctx.enter_context(nc.sbuf_tensor([128, 2], f32))
    xch_r = ctx.enter_context(nc.sbuf_tensor([128, 16], f32))
    kvec = ctx.enter_context(nc.sbuf_tensor([128, 1], f32))
    bvec = ctx.enter_context(nc.sbuf_tensor([128, 1], f32))
    t0v = ctx.enter_context(nc.sbuf_tensor([128, 1], f32))
    t1v = ctx.enter_context(nc.sbuf_tensor([128, 1], f32))
    t2v = ctx.enter_context(nc.sbuf_tensor([128, 1], f32))
    t2av = ctx.enter_context(nc.sbuf_tensor([128, 1], f32))
    s2v = ctx.enter_context(nc.sbuf_tensor([128, 2], f32))
    # one full 2KB PSUM bank per tile: concurrent PE-write + ACT-read in the
    # same bank is a hardware fault, so never co-locate two tiles in a bank.
    ps_full = [ctx.enter_context(nc.psum_tensor(f"ps{i}", [128, 512], f32))
               for i in range(4)]
    ps = [p[:, 0:128] for p in ps_full]
    ps_dummy = ctx.enter_context(nc.psum_tensor("psd", [128, 512], f32))

    # semaphores
    ld = Sem(nc, "ld"); xs = Sem(nc, "xs"); mm = Sem(nc, "mm")
    ynS = Sem(nc, "ynS"); bn = Sem(nc, "bn")
    gd4 = [Sem(nc, f"gd{q}") for q in range(NQUEUE)]
    pq4 = [Sem(nc, f"pq{q}") for q in range(NQUEUE)]
    gq = Sem(nc, "gq"); rs = Sem(nc, "rs"); ls = Sem(nc, "ls")
    dn = Sem(nc, "dn"); dl = Sem(nc, "dl"); psm = Sem(nc, "psm")
    srs = Sem(nc, "srs"); sls = Sem(nc, "sls"); sqr = Sem(nc, "sqr")
    kb = Sem(nc, "kb"); st = Sem(nc, "st"); sq = Sem(nc, "sq")
    od = Sem(nc, "od"); fv = Sem(nc, "fv"); fa = Sem(nc, "fa")

    # sub-broadcast split points (tiles): each fires as soon as its ACT
    # copies land, overlapping the matmul/ACT pipeline.
    bsplits = [16, 32, NT]

    # per-chunk A/B column counts and idx column offsets
    chunk_meta = []
    icol = 0
    for (ta, tb) in chunks:
        colsA = int(sum(128 * SA[t] for t in range(ta, tb)))
        colsB = int(sum(128 * SB[t] for t in range(ta, tb)))
        chunk_meta.append((ta, tb, colsA, colsB, icol, icol + colsA // 16))
        icol += (colsA + colsB) // 16
    assert icol == idx_cols

    # split each chunk's A+B gather into ~equal column spans: each dma_gather
    # auto-fires its descriptors at generation end, so smaller pieces pipeline
    # desc-gen with DMA drain on its SWDGE ring.
    chunk_pieces = []          # per chunk: list of (p0, p1, isA)
    for (ta, tb, colsA, colsB, ic0, icA) in chunk_meta:
        total = colsA + colsB
        marks = sorted({colsA, total} |
                       {min((q * total // NQ) // 128 * 128, total)
                        for q in range(1, NQ)})
        pieces = []
        p0 = 0
        for m in marks:
            if m > p0:
                pieces.append((p0, m, m <= colsA))
                p0 = m
        chunk_pieces.append(pieces)
    # cumulative drain target per global chunk on its queue (per-queue ring:
    # FIFO completion within a queue only)
    gd_cum = []
    qrun = [0] * NQUEUE
    for c in range(layers * nchunks):
        qrun[qof_g[c]] += 16 * len(chunk_pieces[c % nchunks])
        gd_cum.append(qrun[qof_g[c]])

    with nc.Block() as block:

        @block.sync
        def _(sp):
            for d_, s_ in [(idx_sb, idx_d), (drep, drep_d), (dnode, dnode_d),
                           (wsb, wall_d), (gbv, gb_d)]:
                sp.dma_start(d_[:], s_[:]).then_inc(ld.h, 16)
            ld.n = 80
            # layer-0 input loads straight into acc: the aggregation's first
            # write to acc is ordered after every layer-0 transform read.
            sp.dma_start(acc[:], xT_d[:]).then_inc(xs.h, 16)
            xs.n += 16
            if debug_dump:
                sp.wait_ge(kb.h, layers)
                if do_stats:
                    sp.wait_ge(sqr.h, min(layers, 3))
                sp.dma_start(dbg_stage[:], stage[:]).then_inc(od.h, 16)
                od.n += 16
                sp.dma_start(dbg_yn[:], yn[:]).then_inc(od.h, 16)
                od.n += 16
                with nc.allow_non_contiguous_dma(reason="debug dumps"):
                    for j, src_ap in enumerate([xch_r[:], xch_s[:], mv[:],
                                                kvec[:], bvec[:], t0v[:],
                                                t1v[:], s2v[:], stats6[:]]):
                        w = src_ap.shape[1]
                        sp.dma_start(dbg_g.bitcast(f32)[:, 40*j:40*j+w],
                                     src_ap).then_inc(od.h, 16)
                        od.n += 16
            sp.wait_ge(bn.h, layers if (do_stats and layers == 4) else 0)
            if not (do_stats and layers == 4):
                sp.wait_ge(kb.h, layers)
            sp.dma_start(out_d[:], acc[0:64, :]).then_inc(od.h, 16)
            od.n += 16
            sp.wait_ge(od.h, od.n)

        @block.tensor
        def _(te):
            te.wait_ge(ld.h, 80)
            for l in range(layers):
                for t in range(NT):
                    i = l * NT + t
                    if l == 0:
                        if t == 0:
                            te.wait_ge(xs.h, 16)
                        lhsT = acc[:, t * 128:(t + 1) * 128]
                    else:
                        if t == 0:
                            te.wait_ge(bn.h, l)
                        lhsT = acc[:, t * 128:(t + 1) * 128]
                    if i >= 4:
                        te.wait_ge(ynS.h, i - 3)
                    nc.tensor.matmul(
                        ps[i % 4], lhsT,
                        wsb[:, l * 128:(l + 1) * 128],
                        start=True, stop=True,
                    ).then_inc(mm.h, 1)
                    mm.n += 1
                # two per-layer dummy matmuls: the ACT copy of tile i waits
                # mm >= i+2 (PE drain provably complete); the layer's last
                # tiles need successors that don't depend on later layers.
                for _ in range(2):
                    nc.tensor.matmul(
                        ps_dummy[:, 0:128], wsb[:, 0:128], wsb[:, 0:128],
                        start=True, stop=True,
                    ).then_inc(mm.h, 1)
                    mm.n += 1

        @block.scalar
        def _(sc):
            sc.wait_ge(ld.h, 80)
            for l in range(layers):
                for t in range(NT):
                    i = l * NT + t
                    sc.wait_ge(mm.h, l * (NT + 2) + t + 2)
                    if l >= 1 and t == 0:
                        sc.wait_ge(ls.h, 16 * len(bsplits) * l)
                    sc.activation(
                        stage[:, t * 128:(t + 1) * 128], ps[i % 4],
                        AF.Copy, bias=0.0, scale=dnode[:, t:t + 1],
                    ).then_inc(ynS.h, 1)
                    ynS.n += 1
                if not do_stats:
                    continue
                if l < 3:
                    sc.wait_ge(sq.h, l + 1)
                    sc.activation(t1v[:], t0v[:], AF.Sqrt).then_inc(fa.h, 1)
                    fa.n += 1
                    sc.wait_ge(fa.h, fa.n)
                    # readback after fence: t1v committed before sqr fires
                    sc.activation(t2av[:], t1v[:], AF.Copy).then_inc(sqr.h, 1)
                    sqr.n += 1
                    if debug_dump and l == layers - 1:
                        continue
                    sc.wait_ge(kb.h, l + 1)
                    sc.activation(acc[:], acc[:], AF.Relu,
                                  bias=bvec[:], scale=kvec[:],
                                  ).then_inc(bn.h, 1)
                else:
                    sc.wait_ge(kb.h, l + 1)
                    sc.activation(acc[:], acc[:], AF.Identity,
                                  bias=gbv[:, 6:7], scale=1.0,
                                  ).then_inc(bn.h, 1)
                bn.n += 1

        @block.vector
        def _(ve):
            ve.wait_ge(ld.h, 80)
            cidx = 0
            for l in range(layers):
                # Small (4-8B/partition) DVE writes commit lazily: a
                # dependent read in the very next op sees stale data.
                # Fence each small write with a self-semaphore wait.
                def ff(inst):
                    inst.then_inc(fv.h, 1)
                    fv.n += 1
                    ve.wait_ge(fv.h, fv.n)
                # NOTE: no 2-port-perf-mode DVE ops (tensor_tensor with a
                # bf16 operand, bn_stats on f32) may run inside this loop:
                # they grab the shared SBUF port pair that GPSIMD needs to
                # write SWDGE gather descriptors, starving the gather DMA.
                # fp32 tensor_reduce / fp32+fp32 tensor_tensor are 1x-mode
                # (dedicated ports only) and safe.
                for ci, (ta, tb, colsA, colsB, ic0, icA) in enumerate(chunk_meta):
                    if not do_gather:
                        break
                    ve.wait_ge(gd4[qof_g[cidx]].h, gd_cum[cidx])
                    g = G[cidx % NGBUF]
                    offA = 0
                    offB = int(sum(128 * SA[t] for t in range(ta, tb)))
                    for t in range(ta, tb):
                        wA = 128 * int(SA[t])
                        wB = 128 * int(SB[t])
                        ve.tensor_reduce(
                            out=accA[:],
                            in_=g[:, offA:offA + wA].rearrange(
                                "p (n s) -> p n s", n=128),
                            axis=mybir.AxisListType.X, op=AL.add)
                        ve.tensor_reduce(
                            out=accB[:],
                            in_=g[:, offB:offB + wB].rearrange(
                                "p (n s) -> p n s", n=128),
                            axis=mybir.AxisListType.X, op=AL.add)
                        offA += wA
                        offB += wB
                        tt = ve.tensor_tensor(
                            out=acc[:, t * 128:(t + 1) * 128],
                            in0=accA[:],
                            in1=accB[:], op=AL.add)
                    tt.then_inc(gq.h, 1)
                    gq.n += 1
                    # per-chunk dinv_dst scale + BN stats window: pulls this
                    # work off the serial end-of-layer tail, overlapping the
                    # remaining chunks' drains.
                    c0, c1 = ta * 128, tb * 128
                    last_dmul = ve.tensor_tensor(
                        out=acc[:, c0:c1], in0=acc[:, c0:c1],
                        in1=drep[:, c0:c1], op=AL.mult)
                    if do_stats and l < 3:
                        ins_ = ve.bn_stats(stats6[:, ci * 6:(ci + 1) * 6],
                                           acc[:, c0:min(c1, REAL)])
                    cidx += 1
                if do_stats and l < 3:
                    ff(ins_)
                    ff(ve.bn_aggr(mv[:, 0:2], stats6[:, 0:6 * nchunks]))
                    # xch_s = [mean, mean^2 + var] = [Ex, Ex2]
                    if l > 0:
                        ve.wait_ge(sls.h, 16 * l)  # prev stats send done
                    ve.tensor_copy(xch_s[:, 0:1], mv[:, 0:1])
                    ff(ve.tensor_tensor(out=t2v[:], in0=mv[:, 0:1],
                                        in1=mv[:, 0:1], op=AL.mult))
                    ff(ve.tensor_tensor(out=xch_s[:, 1:2], in0=mv[:, 1:2],
                                        in1=t2v[:], op=AL.add))
                    # readback signals xch_s committed
                    ve.tensor_copy(t2v[:], xch_s[:, 0:1]).then_inc(st.h, 1)
                    st.n += 1
                    ve.wait_ge(srs.h, 16 * (l + 1))
                    # global stats: average 8 partials
                    ff(ve.tensor_reduce(
                        out=s2v[:],
                        in_=xch_r[:].rearrange("p (c k) -> p k c", c=8),
                        axis=mybir.AxisListType.X, op=AL.add))
                    ff(ve.tensor_scalar(out=s2v[:], in0=s2v[:],
                                        scalar1=0.125, scalar2=None,
                                        op0=AL.mult))
                    # var = Ex2m - gmean^2 + eps ; t0 = 1/var
                    ff(ve.tensor_tensor(out=t2v[:], in0=s2v[:, 0:1],
                                        in1=s2v[:, 0:1], op=AL.mult))
                    ff(ve.tensor_tensor(out=t0v[:], in0=s2v[:, 1:2],
                                        in1=t2v[:], op=AL.subtract))
                    ff(ve.tensor_scalar(out=t0v[:], in0=t0v[:],
                                        scalar1=BN_EPS, scalar2=None,
                                        op0=AL.add))
                    ff(ve.reciprocal(t0v[:], t0v[:]))
                    ve.tensor_copy(t2v[:], t0v[:]).then_inc(sq.h, 1)
                    sq.n += 1
                    # ACT computes t1 = sqrt(t0) = rstd
                    ve.wait_ge(sqr.h, l + 1)
                    ff(ve.tensor_tensor(out=kvec[:],
                                        in0=gbv[:, 2 * l:2 * l + 1],
                                        in1=t1v[:], op=AL.mult))
                    ff(ve.tensor_tensor(out=t2v[:], in0=s2v[:, 0:1],
                                        in1=kvec[:], op=AL.mult))
                    ff(ve.tensor_tensor(out=bvec[:],
                                        in0=gbv[:, 2 * l + 1:2 * l + 2],
                                        in1=t2v[:], op=AL.subtract))
                    ve.tensor_copy(t2v[:], bvec[:]).then_inc(kb.h, 1)
                else:
                    last_dmul.then_inc(kb.h, 1)
                kb.n += 1

        @block.gpsimd
        def _(gp):
            gp.wait_ge(ld.h, 80)
            gp.memset(yn[:, 0:128], 0)
            gp.memset(yn[:, B_VIEW_RANK * 128 + 32768 - 128:
                          B_VIEW_RANK * 128 + 32768], 0)
            nch = nchunks
            prep_tgt = {}
            npieces = {}

            def emit_prep(li, ci):
                cdx = li * nch + ci
                q = qof_g[cdx]
                g = G[cdx % NGBUF]
                (ta, tb, colsA, colsB, ic0, icA) = chunk_meta[ci]
                cnt = 0
                for (p0, p1, isA) in chunk_pieces[ci]:
                    if isA:
                        in_ap = yn[:, 0:32768]
                        i0 = ic0 + p0 // 16
                    else:
                        in_ap = yn[:, B_VIEW_RANK * 128:
                                   B_VIEW_RANK * 128 + 32768]
                        i0 = icA + (p0 - colsA) // 16
                    w = p1 - p0
                    gp.dma_gather(
                        out_ap=g[:, p0:p1].rearrange(
                            "p (o n) -> p o n", o=1),
                        in_ap=in_ap,
                        idxs_ap=idx_sb[:, i0:i0 + w // 16],
                        num_idxs=w, num_idxs_reg=w,
                        elem_size=128, transpose=True,
                        sbuf_tokens_per_rank=128,
                        sbuf_free_dim_per_rank=256,
                        single_packet=False,
                        queue_num=q,
                        prepare_only=True, sem=gd4[q].h,
                    ).then_inc(pq4[q].h, 1)
                    pq4[q].n += 1
                    gd4[q].n += 16
                    cnt += 1
                prep_tgt[cdx] = pq4[q].n
                npieces[cdx] = cnt

            def emit_trigger(li, ci):
                cdx = li * nch + ci
                q = qof_g[cdx]
                gp.wait_ge(pq4[q].h, prep_tgt[cdx])
                if cdx >= NGBUF:
                    gp.wait_ge(gq.h, cdx - (NGBUF - 1))
                if cdx >= 1:
                    gp.wait_ge(gd4[qof_g[cdx - 1]].h, gd_cum[cdx - 1])
                gp.trigger_dma(count=npieces[cdx], queue_num=q)

            for l in range(layers):
                if l > 0:
                    gp.wait_ge(dn.h, 16 * l)
                prev = 0
                for bs in bsplits:
                    gp.wait_ge(ynS.h, NT * l + bs)
                    ynoff = gp.partition_id() * SHARD + (128 + prev * 128)
                    gp.remote_dma_broadcast(
                        out_ap=yn[:, bass.ds(ynoff, (bs - prev) * 128)],
                        in_ap=stage[:, prev * 128:bs * 128],
                        remote_sem=rs.h, local_sem=ls.h,
                        rdests=[(0, k) for k in range(NCORES)],
                    ).then_inc(psm.h, 1)
                    psm.n += 1
                    gp.wait_ge(psm.h, psm.n)
                    gp.trigger_dma(count=1)
                    prev = bs
                if do_gather:
                    # layer 0's first preps emit here; later layers' were
                    # already emitted at the end of the previous layer so
                    # their desc-gen hides under the layer boundary.
                    if l == 0:
                        for ci in range(min(LOOKAHEAD, nch)):
                            emit_prep(0, ci)
                    gp.wait_ge(rs.h, 16 * len(bsplits) * (l + 1))
                    for ci in range(nch):
                        emit_trigger(l, ci)
                        if ci + LOOKAHEAD < nch:
                            emit_prep(l, ci + LOOKAHEAD)
                    # this layer's drain totals, captured BEFORE the next
                    # layer's early preps bump the counters (their sems only
                    # fire after the next layer's triggers).
                    gd_tgt = [gd4[q].n for q in range(NQUEUE)]
                    if l + 1 < layers:
                        for ci in range(min(LOOKAHEAD, nch)):
                            emit_prep(l + 1, ci)
                else:
                    gp.wait_ge(rs.h, 16 * len(bsplits) * (l + 1))
                    gd_tgt = [gd4[q].n for q in range(NQUEUE)]
                for q in range(NQUEUE):
                    gp.wait_ge(gd4[q].h, gd_tgt[q])
                gp.remote_sem_update_broadcast(
                    remote_sem=dn.h, local_sem=dl.h,
                    rdests=[(0, k) for k in range(NCORES)],
                ).then_inc(psm.h, 1)
                psm.n += 1
                gp.wait_ge(psm.h, psm.n)
                gp.trigger_dma(count=1)
                if do_stats and l < 3:
                    gp.wait_ge(st.h, l + 1)
                    xoff = gp.partition_id() * 2
                    gp.remote_dma_broadcast(
                        out_ap=xch_r[:, bass.ds(xoff, 2)],
                        in_ap=xch_s[:],
                        remote_sem=srs.h, local_sem=sls.h,
                        rdests=[(0, k) for k in range(NCORES)],
                    ).then_inc(psm.h, 1)
                    psm.n += 1
                    gp.wait_ge(psm.h, psm.n)
                    gp.trigger_dma(count=1)

    nc.compile()
    return nc


def make_core_inputs(pp, x, Ws, gb):
    """per-core in_maps for run_bass_kernel_spmd / run_bass_via_pjrt"""
    nos = pp["node_of_slot"]
    dinv_slot = pp["dinv_slot"]
    wall = np.zeros((128, 512), np.float32)
    wall[:, 0:128] = Ws[0]
    wall[:, 128:256] = Ws[1]
    wall[:, 256:384] = Ws[2]
    wall[:, 384:448] = Ws[3][:, :64] if Ws[3].shape[1] == 64 else Ws[3][:, :]
    in_maps = []
    for c in range(NCORES):
        slots = c * SHARD + np.arange(SHARD)
        nodes = nos[slots]
        msk = nodes >= 0
        xT = np.zeros((128, SHARD), np.float32)
        xT[:, msk] = x[nodes[msk]].T
        drep = np.broadcast_to(
            dinv_slot[slots].astype(bf16), (128, SHARD)).copy()
        dnode = dinv_slot[slots].reshape(NT, 128).T.copy().astype(np.float32)
        in_maps.append(dict(xT=xT, idx=pp["idx_dram"][c].copy(),
                            drep=drep, dnode=dnode, wall=wall.copy(),
                            gb=gb.copy()))
    return in_maps


def make_gb(g1, be1, g2, be2, g3, be3, b4):
    gb = np.zeros((128, 8), np.float32)
    for i, v in enumerate([g1, be1, g2, be2, g3, be3]):
        gb[:, i] = v
    gb[:64, 6] = b4
    return gb


def assemble_output(pp, results):
    nos = pp["node_of_slot"]
    full = np.zeros((N, OUT), np.float32)
    for c in range(NCORES):
        slots = c * SHARD + np.arange(SHARD)
        nodes = nos[slots]
        msk = nodes >= 0
        full[nodes[msk]] = results[c]["out"][:OUT, msk].T
    return full


# ---------------------------------------------------------------------------
# public entry point
# ---------------------------------------------------------------------------
_CACHE = {}


def _get_program(edge_index):
    key = hash(edge_index.tobytes())
    if key not in _CACHE:
        pp = preprocess(edge_index)
        nc = build_program(pp)
        _CACHE[key] = (pp, nc)
    return _CACHE[key]


def kernel(**inputs):
    """Full GCN encoder on 8 TRN2 NeuronCores.

    Takes the full (unsharded) inputs of reference.setup_inputs(), returns
    the full [50000, 64] float32 output.
    """
    from concourse import bass2jax

    inputs = {k: np.asarray(v) for k, v in inputs.items()}
    edge_index = inputs["edge_index"].astype(np.int32)
    pp, nc = _get_program(edge_index)
    Ws = [inputs["W1"], inputs["W2"], inputs["W3"], inputs["W4"]]
    gb = make_gb(inputs["g1"], inputs["be1"], inputs["g2"], inputs["be2"],
                 inputs["g3"], inputs["be3"], inputs["b4"])
    # bias handling: b1..b3 cancel inside batch-norm (per-feature constant
    # shifts drop out of x - mean); b4 is applied on-device via gb col 6.
    in_maps = make_core_inputs(pp, inputs["x"].astype(np.float32), Ws, gb)
    results = bass2jax.run_bass_via_pjrt(nc, in_maps, n_cores=NCORES)
    return assemble_output(pp, results)

